# revision 14
# baseline (speedup 1.0000x reference)
"""Trainium2 kernel for nn_Detect (decode + score + threshold + top-k + NMS).

Strategy (memory-regime):
  - The dominant traffic is box_prob [1,32768,8,80] f32 (84MB). Shard the
    262144 candidates across 8 NeuronCores (32768 per core).
  - Each core computes score[c] = box_conf[c] * max_cls(box_prob[c, :]) and
    applies the score threshold (score if > 0.4 else -1) — one streaming,
    DMA-bound pass. This equals max_cls(conf*prob) bit-exactly because fp32
    multiply by a non-negative scalar is monotone under round-to-nearest.
  - Host gathers the 262144 scores, selects the global top-200 (stable,
    lower-index-first on ties, matching jax.lax.top_k), decodes boxes and
    argmax classes for just those 200 rows, and runs the 200x200 greedy NMS
    + compaction exactly as the reference does.
"""

import contextlib
import sys

for _p in ("/opt/trn_rl_repo",):
    if _p not in sys.path:
        sys.path.insert(0, _p)

import numpy as np

import concourse.bass as bass
import concourse.bacc as bacc
import concourse.mybir as mybir
from concourse import tile
from concourse.bass_utils import run_bass_kernel_spmd

# Problem constants (hardcoded per harness contract).
N_CORES = 8
N_FULL = 262144          # 1 * 32768 * 8 candidates
N_CORE = N_FULL // N_CORES  # 32768 candidates per core
C = 80                   # classes
P = 128                  # SBUF partitions
FP = N_CORE // P         # 256 candidates per partition
F = 32                   # candidates-per-partition per streamed chunk
SCORE_T = np.float32(0.4)
NMS_T = np.float32(0.4)
TOP_K = 200

_nc_cache = None


def _build_nc() -> bass.Bass:
    """Per-core program: scores[p,f] = thresholded conf*max_cls(prob).

    Raw bacc (no TileContext): hand-placed semaphores avoid Tile's ~3.5us
    startup and ~6us tail all-engine barriers. Bacc.compile() still splits
    any multi-semaphore wait (TRN2 allows one wait per instruction).
    """
    nc = bacc.Bacc(None, target_bir_lowering=False)
    f32 = mybir.dt.float32
    NCH = FP // F
    # Candidate c (0..32767) lives at partition c//FP, free slot c%FP, so all
    # DMAs are contiguous per partition.
    prob = nc.dram_tensor("prob", [P, FP, C], f32, kind="ExternalInput")
    conf = nc.dram_tensor("conf", [P, FP], f32, kind="ExternalInput")
    scores = nc.dram_tensor("scores", [P, FP], f32, kind="ExternalOutput")

    with (
        nc.Block() as block,
        nc.sbuf_tensor("conf_t", [P, FP], f32) as conf_t,
        nc.sbuf_tensor("maxp", [P, FP], f32) as maxp,
        nc.sbuf_tensor("s_t", [P, FP], f32) as s_t,
        nc.sbuf_tensor("mask_t", [P, FP], mybir.dt.uint8) as mask_t,
        nc.sbuf_tensor("res_t", [P, FP], f32) as res_t,
        nc.semaphore("conf_sem") as conf_sem,
        nc.semaphore("dma_sem") as dma_sem,
        nc.semaphore("dve_sem") as dve_sem,
        nc.semaphore("out_sem") as out_sem,
        contextlib.ExitStack() as st,
    ):
        chunks = [
            st.enter_context(nc.sbuf_tensor(f"chunk{i}", [P, F, C], f32))
            for i in range(NCH)
        ]

        @block.sync
        def _(sync):
            sync.dma_start(conf_t[:, :], conf[:, :]).then_inc(conf_sem, 16)
            for i in range(NCH):
                sync.dma_start(
                    chunks[i][:, :, :], prob[:, i * F : (i + 1) * F, :]
                ).then_inc(dma_sem, 16)
            sync.wait_ge(dve_sem, NCH + 4)
            sync.dma_start(scores[:, :], res_t[:, :]).then_inc(out_sem, 16)
            sync.wait_ge(out_sem, 16)

        @block.vector
        def _(vector):
            nc.vector.memset(res_t[:, :], -1.0).then_inc(dve_sem, 1)
            for i in range(NCH):
                vector.wait_ge(dma_sem, 16 * (i + 1))
                nc.vector.reduce_max(
                    maxp[:, i * F : (i + 1) * F],
                    chunks[i][:, :, :],
                    axis=mybir.AxisListType.X,
                ).then_inc(dve_sem, 1)
            vector.wait_ge(conf_sem, 16)
            # Same-engine self-waits guard DVE pipeline RAW hazards (mirrors
            # the waits Tile emits).
            vector.wait_ge(dve_sem, NCH + 1)
            nc.vector.tensor_mul(s_t[:, :], maxp[:, :], conf_t[:, :]).then_inc(
                dve_sem, 1
            )
            vector.wait_ge(dve_sem, NCH + 2)
            nc.vector.tensor_scalar(
                mask_t[:, :], s_t[:, :], float(SCORE_T), None,
                op0=mybir.AluOpType.is_gt,
            ).then_inc(dve_sem, 1)
            vector.wait_ge(dve_sem, NCH + 3)
            nc.vector.copy_predicated(res_t[:, :], mask_t[:, :], s_t[:, :]).then_inc(
                dve_sem, 1
            )

    nc.finalize()
    return nc


def _get_nc() -> bass.Bass:
    global _nc_cache
    if _nc_cache is None:
        _nc_cache = _build_nc()
    return _nc_cache


def _device_scores(box_conf: np.ndarray, box_prob: np.ndarray, trace: bool = False):
    """Run the SPMD score pass on 8 cores; returns (scores[262144], results)."""
    prob_flat = np.ascontiguousarray(box_prob.reshape(N_FULL, C))
    conf_flat = np.ascontiguousarray(box_conf.reshape(N_FULL))
    in_maps = [
        {
            "prob": prob_flat[k * N_CORE : (k + 1) * N_CORE].reshape(P, FP, C),
            "conf": conf_flat[k * N_CORE : (k + 1) * N_CORE].reshape(P, FP),
        }
        for k in range(N_CORES)
    ]
    res = run_bass_kernel_spmd(
        _get_nc(), in_maps, core_ids=list(range(N_CORES)), trace=trace
    )
    scores = np.concatenate(
        [res.results[k]["scores"].reshape(N_CORE) for k in range(N_CORES)]
    )
    return scores, res


def _postprocess(scores, box_pred, box_conf, box_prob, priors, img_shape):
    """Bit-exact replication of the reference from the score tensor onward."""
    # --- top-200, stable lower-index-first on ties (jax.lax.top_k semantics)
    superset = 1024
    part = np.argpartition(-scores, superset)[:superset]
    part = part[np.lexsort((part, -scores[part]))]
    top_idx = part[:TOP_K]
    top_scores = scores[top_idx]

    # --- decode boxes for the selected rows only (elementwise ops commute
    # with the gather; all ops IEEE f32 exactly as the reference)
    pred = box_pred.reshape(N_FULL, 4)[top_idx]
    pri = priors.reshape(N_FULL, 4)[top_idx]
    xy = pred[:, 0:2] + pri[:, 0:2]
    wh = pred[:, 2:4] * pri[:, 2:4]
    half = np.float32(0.5) * wh
    corners = np.concatenate([xy - half, xy + half], axis=1)
    corners = corners / np.array([30.0, 10.0, 30.0, 10.0], np.float32)
    top_boxes = corners * img_shape.reshape(1, 4)

    # --- classes for the selected rows (argmax of conf*prob, first-max ties)
    all_scores = box_conf.reshape(N_FULL, 1)[top_idx] * box_prob.reshape(N_FULL, C)[top_idx]
    top_classes = np.argmax(all_scores, axis=1).astype(np.int32)

    # --- greedy NMS over the fixed top-200 (replicates _greedy_nms_keep)
    valid = top_scores > SCORE_T
    x1, y1, x2, y2 = top_boxes[:, 0], top_boxes[:, 1], top_boxes[:, 2], top_boxes[:, 3]
    area = (x2 - x1) * (y2 - y1)
    xx1 = np.maximum(x1[:, None], x1[None, :])
    yy1 = np.maximum(y1[:, None], y1[None, :])
    xx2 = np.minimum(x2[:, None], x2[None, :])
    yy2 = np.minimum(y2[:, None], y2[None, :])
    inter = np.clip(xx2 - xx1, 0.0, None) * np.clip(yy2 - yy1, 0.0, None)
    with np.errstate(divide="ignore", invalid="ignore"):
        iou = inter / (area[:, None] + area[None, :] - inter)
    idxs = np.arange(TOP_K)
    keep = valid.copy()
    for i in range(TOP_K):
        sup = keep[i] & (iou[i] > NMS_T) & (idxs > i)
        keep = keep & ~sup

    # --- compact kept detections to the front, zero-pad rest
    order = np.argsort(np.where(keep, 0, 1), kind="stable")
    kmask = keep[order]
    out_boxes = np.where(kmask[:, None], top_boxes[order], np.float32(0.0))
    out_scores = np.where(kmask, top_scores[order], np.float32(0.0))
    out_classes = np.where(kmask, top_classes[order], -1).astype(np.int32)
    count = np.int32(keep.sum())
    return out_boxes, out_scores, out_classes, count


def kernel(box_pred, box_conf, box_prob, priors, img_shape):
    box_pred = np.asarray(box_pred, dtype=np.float32)
    box_conf = np.asarray(box_conf, dtype=np.float32)
    box_prob = np.asarray(box_prob, dtype=np.float32)
    priors = np.asarray(priors, dtype=np.float32)
    img_shape = np.asarray(img_shape, dtype=np.float32)

    scores, _ = _device_scores(box_conf, box_prob)
    return _postprocess(scores, box_pred, box_conf, box_prob, priors, img_shape)


# revision 17
# speedup vs baseline: 1.0056x; 1.0056x over previous
"""Trainium2 kernel for nn_Detect (decode + score + threshold + top-k + NMS).

Strategy (memory-regime):
  - The dominant traffic is box_prob [1,32768,8,80] f32 (84MB). Shard the
    262144 candidates across 8 NeuronCores (32768 per core).
  - Each core computes score[c] = box_conf[c] * max_cls(box_prob[c, :]) and
    applies the score threshold (score if > 0.4 else -1) — one streaming,
    DMA-bound pass. This equals max_cls(conf*prob) bit-exactly because fp32
    multiply by a non-negative scalar is monotone under round-to-nearest.
  - Host gathers the 262144 scores, selects the global top-200 (stable,
    lower-index-first on ties, matching jax.lax.top_k), decodes boxes and
    argmax classes for just those 200 rows, and runs the 200x200 greedy NMS
    + compaction exactly as the reference does.
"""

import contextlib
import sys

for _p in ("/opt/trn_rl_repo",):
    if _p not in sys.path:
        sys.path.insert(0, _p)

import numpy as np

import concourse.bass as bass
import concourse.bacc as bacc
import concourse.mybir as mybir
from concourse import tile
from concourse.bass_utils import run_bass_kernel_spmd

# Problem constants (hardcoded per harness contract).
N_CORES = 8
N_FULL = 262144          # 1 * 32768 * 8 candidates
N_CORE = N_FULL // N_CORES  # 32768 candidates per core
C = 80                   # classes
P = 128                  # SBUF partitions
FP = N_CORE // P         # 256 candidates per partition
F = 16                   # candidates-per-partition per streamed chunk
GRP = 4                  # chunks per elementwise/output group
SCORE_T = np.float32(0.4)
NMS_T = np.float32(0.4)
TOP_K = 200

_nc_cache = None


def _build_nc() -> bass.Bass:
    """Per-core program: scores[p,f] = thresholded conf*max_cls(prob).

    Raw bacc (no TileContext): hand-placed semaphores avoid Tile's ~3.5us
    startup and ~6us tail all-engine barriers. Bacc.compile() still splits
    any multi-semaphore wait (TRN2 allows one wait per instruction).
    """
    nc = bacc.Bacc(None, target_bir_lowering=False, enable_partition_id=False)
    f32 = mybir.dt.float32
    NCH = FP // F
    # Candidate c (0..32767) lives at partition c//FP, free slot c%FP, so all
    # DMAs are contiguous per partition.
    prob = nc.dram_tensor("prob", [P, FP, C], f32, kind="ExternalInput")
    conf = nc.dram_tensor("conf", [P, FP], f32, kind="ExternalInput")
    scores = nc.dram_tensor("scores", [P, FP], f32, kind="ExternalOutput")

    with (
        nc.Block() as block,
        nc.sbuf_tensor("conf_t", [P, FP], f32) as conf_t,
        nc.sbuf_tensor("maxp", [P, FP], f32) as maxp,
        nc.sbuf_tensor("s_t", [P, FP], f32) as s_t,
        nc.sbuf_tensor("mask_t", [P, FP], mybir.dt.uint8) as mask_t,
        nc.sbuf_tensor("res_t", [P, FP], f32) as res_t,
        nc.semaphore("conf_sem") as conf_sem,
        nc.semaphore("dma_sem") as dma_sem,
        nc.semaphore("dve_sem") as dve_sem,
        nc.semaphore("out_sem") as out_sem,
        contextlib.ExitStack() as st,
    ):
        chunks = [
            st.enter_context(nc.sbuf_tensor(f"chunk{i}", [P, F, C], f32))
            for i in range(NCH)
        ]

        n_groups = NCH // GRP
        GF = GRP * F                  # candidates-per-partition per group
        # DVE completion count just after each group's copy_predicated:
        # memset(1) + per group: GRP reduces + mul + is_gt + copy_pred.
        def dve_after_group(g):
            return 1 + (g + 1) * (GRP + 3)

        @block.sync
        def _(sync):
            # First GRP chunk loads, then conf (not needed until group 0's
            # mul), then the rest of the stream.
            for i in range(GRP):
                sync.dma_start(
                    chunks[i][:, :, :], prob[:, i * F : (i + 1) * F, :]
                ).then_inc(dma_sem, 16)
            sync.dma_start(conf_t[:, :], conf[:, :]).then_inc(conf_sem, 16)
            for i in range(GRP, NCH):
                sync.dma_start(
                    chunks[i][:, :, :], prob[:, i * F : (i + 1) * F, :]
                ).then_inc(dma_sem, 16)
            # First half of the scores goes out as soon as groups 0..1 are
            # final; the tail only pays for the second half.
            half = n_groups // 2
            sync.wait_ge(dve_sem, dve_after_group(half - 1))
            sync.dma_start(
                scores[:, 0 : half * GF], res_t[:, 0 : half * GF]
            ).then_inc(out_sem, 16)
            sync.wait_ge(dve_sem, dve_after_group(n_groups - 1))
            sync.dma_start(
                scores[:, half * GF : FP], res_t[:, half * GF : FP]
            ).then_inc(out_sem, 16)
            sync.wait_ge(out_sem, 32)

        @block.vector
        def _(vector):
            dve = 0
            nc.vector.memset(res_t[:, :], -1.0).then_inc(dve_sem, 1)
            dve += 1
            for g in range(n_groups):
                for i in range(g * GRP, (g + 1) * GRP):
                    vector.wait_ge(dma_sem, 16 * (i + 1))
                    nc.vector.reduce_max(
                        maxp[:, i * F : (i + 1) * F],
                        chunks[i][:, :, :],
                        axis=mybir.AxisListType.X,
                    ).then_inc(dve_sem, 1)
                    dve += 1
                sl = slice(g * GF, (g + 1) * GF)
                if g == 0:
                    vector.wait_ge(conf_sem, 16)
                # Same-engine self-waits guard DVE pipeline RAW hazards
                # (mirrors the waits Tile emits).
                vector.wait_ge(dve_sem, dve)
                nc.vector.tensor_mul(
                    s_t[:, sl], maxp[:, sl], conf_t[:, sl]
                ).then_inc(dve_sem, 1)
                dve += 1
                vector.wait_ge(dve_sem, dve)
                nc.vector.tensor_scalar(
                    mask_t[:, sl], s_t[:, sl], float(SCORE_T), None,
                    op0=mybir.AluOpType.is_gt,
                ).then_inc(dve_sem, 1)
                dve += 1
                vector.wait_ge(dve_sem, dve)
                nc.vector.copy_predicated(
                    res_t[:, sl], mask_t[:, sl], s_t[:, sl]
                ).then_inc(dve_sem, 1)
                dve += 1
                assert dve == dve_after_group(g)

    nc.finalize()
    return nc


def _get_nc() -> bass.Bass:
    global _nc_cache
    if _nc_cache is None:
        _nc_cache = _build_nc()
    return _nc_cache


def _device_scores(box_conf: np.ndarray, box_prob: np.ndarray, trace: bool = False):
    """Run the SPMD score pass on 8 cores; returns (scores[262144], results)."""
    prob_flat = np.ascontiguousarray(box_prob.reshape(N_FULL, C))
    conf_flat = np.ascontiguousarray(box_conf.reshape(N_FULL))
    in_maps = [
        {
            "prob": prob_flat[k * N_CORE : (k + 1) * N_CORE].reshape(P, FP, C),
            "conf": conf_flat[k * N_CORE : (k + 1) * N_CORE].reshape(P, FP),
        }
        for k in range(N_CORES)
    ]
    res = run_bass_kernel_spmd(
        _get_nc(), in_maps, core_ids=list(range(N_CORES)), trace=trace
    )
    scores = np.concatenate(
        [res.results[k]["scores"].reshape(N_CORE) for k in range(N_CORES)]
    )
    return scores, res


def _postprocess(scores, box_pred, box_conf, box_prob, priors, img_shape):
    """Bit-exact replication of the reference from the score tensor onward."""
    # --- top-200, stable lower-index-first on ties (jax.lax.top_k semantics)
    superset = 1024
    part = np.argpartition(-scores, superset)[:superset]
    part = part[np.lexsort((part, -scores[part]))]
    top_idx = part[:TOP_K]
    top_scores = scores[top_idx]

    # --- decode boxes for the selected rows only (elementwise ops commute
    # with the gather; all ops IEEE f32 exactly as the reference)
    pred = box_pred.reshape(N_FULL, 4)[top_idx]
    pri = priors.reshape(N_FULL, 4)[top_idx]
    xy = pred[:, 0:2] + pri[:, 0:2]
    wh = pred[:, 2:4] * pri[:, 2:4]
    half = np.float32(0.5) * wh
    corners = np.concatenate([xy - half, xy + half], axis=1)
    corners = corners / np.array([30.0, 10.0, 30.0, 10.0], np.float32)
    top_boxes = corners * img_shape.reshape(1, 4)

    # --- classes for the selected rows (argmax of conf*prob, first-max ties)
    all_scores = box_conf.reshape(N_FULL, 1)[top_idx] * box_prob.reshape(N_FULL, C)[top_idx]
    top_classes = np.argmax(all_scores, axis=1).astype(np.int32)

    # --- greedy NMS over the fixed top-200 (replicates _greedy_nms_keep)
    valid = top_scores > SCORE_T
    x1, y1, x2, y2 = top_boxes[:, 0], top_boxes[:, 1], top_boxes[:, 2], top_boxes[:, 3]
    area = (x2 - x1) * (y2 - y1)
    xx1 = np.maximum(x1[:, None], x1[None, :])
    yy1 = np.maximum(y1[:, None], y1[None, :])
    xx2 = np.minimum(x2[:, None], x2[None, :])
    yy2 = np.minimum(y2[:, None], y2[None, :])
    inter = np.clip(xx2 - xx1, 0.0, None) * np.clip(yy2 - yy1, 0.0, None)
    with np.errstate(divide="ignore", invalid="ignore"):
        iou = inter / (area[:, None] + area[None, :] - inter)
    idxs = np.arange(TOP_K)
    keep = valid.copy()
    for i in range(TOP_K):
        sup = keep[i] & (iou[i] > NMS_T) & (idxs > i)
        keep = keep & ~sup

    # --- compact kept detections to the front, zero-pad rest
    order = np.argsort(np.where(keep, 0, 1), kind="stable")
    kmask = keep[order]
    out_boxes = np.where(kmask[:, None], top_boxes[order], np.float32(0.0))
    out_scores = np.where(kmask, top_scores[order], np.float32(0.0))
    out_classes = np.where(kmask, top_classes[order], -1).astype(np.int32)
    count = np.int32(keep.sum())
    return out_boxes, out_scores, out_classes, count


def kernel(box_pred, box_conf, box_prob, priors, img_shape):
    box_pred = np.asarray(box_pred, dtype=np.float32)
    box_conf = np.asarray(box_conf, dtype=np.float32)
    box_prob = np.asarray(box_prob, dtype=np.float32)
    priors = np.asarray(priors, dtype=np.float32)
    img_shape = np.asarray(img_shape, dtype=np.float32)

    scores, _ = _device_scores(box_conf, box_prob)
    return _postprocess(scores, box_pred, box_conf, box_prob, priors, img_shape)


# revision 21
# speedup vs baseline: 1.1474x; 1.1410x over previous
"""Trainium2 kernel for nn_Detect (decode + score + threshold + top-k + NMS).

Strategy (memory-regime):
  - The dominant traffic is box_prob [1,32768,8,80] f32 (84MB). Shard the
    262144 candidates across 8 NeuronCores (32768 per core).
  - Each core computes score[c] = box_conf[c] * max_cls(box_prob[c, :]) and
    applies the score threshold (score if > 0.4 else -1) — one streaming,
    DMA-bound pass. This equals max_cls(conf*prob) bit-exactly because fp32
    multiply by a non-negative scalar is monotone under round-to-nearest.
  - Host gathers the 262144 scores, selects the global top-200 (stable,
    lower-index-first on ties, matching jax.lax.top_k), decodes boxes and
    argmax classes for just those 200 rows, and runs the 200x200 greedy NMS
    + compaction exactly as the reference does.
"""

import contextlib
import sys

for _p in ("/opt/trn_rl_repo",):
    if _p not in sys.path:
        sys.path.insert(0, _p)

import numpy as np

import concourse.bass as bass
import concourse.bacc as bacc
import concourse.mybir as mybir
from concourse import tile
from concourse.bass_utils import run_bass_kernel_spmd

# Problem constants (hardcoded per harness contract).
N_CORES = 8
N_FULL = 262144          # 1 * 32768 * 8 candidates
N_CORE = N_FULL // N_CORES  # 32768 candidates per core
C = 80                   # classes
P = 128                  # SBUF partitions
FP = N_CORE // P         # 256 candidates per partition
F = 16                   # candidates-per-partition per streamed chunk
GRP = 4                  # chunks per elementwise/output group
SCORE_T = np.float32(0.4)
NMS_T = np.float32(0.4)
TOP_K = 200

_nc_cache = None


def _build_nc() -> bass.Bass:
    """Per-core program: scores[p,f] = thresholded conf*max_cls(prob).

    Raw bacc (no TileContext): hand-placed semaphores avoid Tile's ~3.5us
    startup and ~6us tail all-engine barriers. Bacc.compile() still splits
    any multi-semaphore wait (TRN2 allows one wait per instruction).
    """
    nc = bacc.Bacc(None, target_bir_lowering=False, enable_partition_id=False)
    f32 = mybir.dt.float32
    NCH = FP // F
    # Candidate c (0..32767) lives at partition c//FP, free slot c%FP, so all
    # DMAs are contiguous per partition.
    prob = nc.dram_tensor("prob", [P, FP, C], f32, kind="ExternalInput")
    conf = nc.dram_tensor("conf", [P, FP], f32, kind="ExternalInput")
    scores = nc.dram_tensor("scores", [P, FP], f32, kind="ExternalOutput")

    with (
        nc.Block(no_gpsimd_drain=True) as block,
        nc.sbuf_tensor("conf_t", [P, FP], f32) as conf_t,
        nc.sbuf_tensor("maxp", [P, FP], f32) as maxp,
        nc.sbuf_tensor("s_t", [P, FP], f32) as s_t,
        nc.sbuf_tensor("mask_t", [P, FP], mybir.dt.uint8) as mask_t,
        nc.sbuf_tensor("res_t", [P, FP], f32) as res_t,
        nc.semaphore("conf_sem") as conf_sem,
        nc.semaphore("dve_sem") as dve_sem,
        nc.semaphore("out_sem") as out_sem,
        contextlib.ExitStack() as st,
    ):
        chunks = [
            st.enter_context(nc.sbuf_tensor(f"chunk{i}", [P, F, C], f32))
            for i in range(NCH)
        ]
        # One semaphore per chunk: a shared counter is NOT safe — the 16 SDMA
        # engines complete a DMA's descriptors in parallel and out of order
        # across queued DMAs, so "total incs >= 16*(i+1)" does not imply
        # chunk i fully landed.
        chunk_sems = [
            st.enter_context(nc.semaphore(f"chunk_sem{i}")) for i in range(NCH)
        ]

        n_groups = NCH // GRP
        GF = GRP * F                  # candidates-per-partition per group
        # DVE completion count just after each group's copy_predicated:
        # memset(1) + per group: GRP reduces + mul + is_gt + copy_pred.
        def dve_after_group(g):
            return 1 + (g + 1) * (GRP + 3)

        @block.sync
        def _(sync):
            # First GRP chunk loads, then conf (not needed until group 0's
            # mul), then the rest of the stream.
            for i in range(GRP):
                sync.dma_start(
                    chunks[i][:, :, :], prob[:, i * F : (i + 1) * F, :]
                ).then_inc(chunk_sems[i], 16)
            sync.dma_start(conf_t[:, :], conf[:, :]).then_inc(conf_sem, 16)
            for i in range(GRP, NCH):
                sync.dma_start(
                    chunks[i][:, :, :], prob[:, i * F : (i + 1) * F, :]
                ).then_inc(chunk_sems[i], 16)

        @block.scalar
        def _(scalar):
            # Stores ride the Activation HWDGE ring (qActDynamicHW), fully
            # decoupled from the SP load ring. First half goes out as soon as
            # groups 0..half-1 are final; the tail only pays for the rest.
            half = n_groups // 2
            scalar.wait_ge(dve_sem, dve_after_group(half - 1))
            scalar.dma_start(
                scores[:, 0 : half * GF], res_t[:, 0 : half * GF]
            ).then_inc(out_sem, 16)
            scalar.wait_ge(dve_sem, dve_after_group(n_groups - 1))
            scalar.dma_start(
                scores[:, half * GF : FP], res_t[:, half * GF : FP]
            ).then_inc(out_sem, 16)
            scalar.wait_ge(out_sem, 32)

        @block.vector
        def _(vector):
            dve = 0
            nc.vector.memset(res_t[:, :], -1.0).then_inc(dve_sem, 1)
            dve += 1
            for g in range(n_groups):
                for i in range(g * GRP, (g + 1) * GRP):
                    vector.wait_ge(chunk_sems[i], 16)
                    nc.vector.reduce_max(
                        maxp[:, i * F : (i + 1) * F],
                        chunks[i][:, :, :],
                        axis=mybir.AxisListType.X,
                    ).then_inc(dve_sem, 1)
                    dve += 1
                sl = slice(g * GF, (g + 1) * GF)
                if g == 0:
                    vector.wait_ge(conf_sem, 16)
                # Same-engine self-waits guard DVE pipeline RAW hazards
                # (mirrors the waits Tile emits).
                vector.wait_ge(dve_sem, dve)
                nc.vector.tensor_mul(
                    s_t[:, sl], maxp[:, sl], conf_t[:, sl]
                ).then_inc(dve_sem, 1)
                dve += 1
                vector.wait_ge(dve_sem, dve)
                nc.vector.tensor_scalar(
                    mask_t[:, sl], s_t[:, sl], float(SCORE_T), None,
                    op0=mybir.AluOpType.is_gt,
                ).then_inc(dve_sem, 1)
                dve += 1
                vector.wait_ge(dve_sem, dve)
                nc.vector.copy_predicated(
                    res_t[:, sl], mask_t[:, sl], s_t[:, sl]
                ).then_inc(dve_sem, 1)
                dve += 1
                assert dve == dve_after_group(g)

    nc.finalize()
    return nc


def _get_nc() -> bass.Bass:
    global _nc_cache
    if _nc_cache is None:
        _nc_cache = _build_nc()
    return _nc_cache


def _device_scores(box_conf: np.ndarray, box_prob: np.ndarray, trace: bool = False):
    """Run the SPMD score pass on 8 cores; returns (scores[262144], results)."""
    prob_flat = np.ascontiguousarray(box_prob.reshape(N_FULL, C))
    conf_flat = np.ascontiguousarray(box_conf.reshape(N_FULL))
    in_maps = [
        {
            "prob": prob_flat[k * N_CORE : (k + 1) * N_CORE].reshape(P, FP, C),
            "conf": conf_flat[k * N_CORE : (k + 1) * N_CORE].reshape(P, FP),
        }
        for k in range(N_CORES)
    ]
    res = run_bass_kernel_spmd(
        _get_nc(), in_maps, core_ids=list(range(N_CORES)), trace=trace
    )
    scores = np.concatenate(
        [res.results[k]["scores"].reshape(N_CORE) for k in range(N_CORES)]
    )
    return scores, res


def _postprocess(scores, box_pred, box_conf, box_prob, priors, img_shape):
    """Bit-exact replication of the reference from the score tensor onward."""
    # --- top-200, stable lower-index-first on ties (jax.lax.top_k semantics)
    superset = 1024
    part = np.argpartition(-scores, superset)[:superset]
    part = part[np.lexsort((part, -scores[part]))]
    top_idx = part[:TOP_K]
    top_scores = scores[top_idx]

    # --- decode boxes for the selected rows only (elementwise ops commute
    # with the gather; all ops IEEE f32 exactly as the reference)
    pred = box_pred.reshape(N_FULL, 4)[top_idx]
    pri = priors.reshape(N_FULL, 4)[top_idx]
    xy = pred[:, 0:2] + pri[:, 0:2]
    wh = pred[:, 2:4] * pri[:, 2:4]
    half = np.float32(0.5) * wh
    corners = np.concatenate([xy - half, xy + half], axis=1)
    corners = corners / np.array([30.0, 10.0, 30.0, 10.0], np.float32)
    top_boxes = corners * img_shape.reshape(1, 4)

    # --- classes for the selected rows (argmax of conf*prob, first-max ties)
    all_scores = box_conf.reshape(N_FULL, 1)[top_idx] * box_prob.reshape(N_FULL, C)[top_idx]
    top_classes = np.argmax(all_scores, axis=1).astype(np.int32)

    # --- greedy NMS over the fixed top-200 (replicates _greedy_nms_keep)
    valid = top_scores > SCORE_T
    x1, y1, x2, y2 = top_boxes[:, 0], top_boxes[:, 1], top_boxes[:, 2], top_boxes[:, 3]
    area = (x2 - x1) * (y2 - y1)
    xx1 = np.maximum(x1[:, None], x1[None, :])
    yy1 = np.maximum(y1[:, None], y1[None, :])
    xx2 = np.minimum(x2[:, None], x2[None, :])
    yy2 = np.minimum(y2[:, None], y2[None, :])
    inter = np.clip(xx2 - xx1, 0.0, None) * np.clip(yy2 - yy1, 0.0, None)
    with np.errstate(divide="ignore", invalid="ignore"):
        iou = inter / (area[:, None] + area[None, :] - inter)
    idxs = np.arange(TOP_K)
    keep = valid.copy()
    for i in range(TOP_K):
        sup = keep[i] & (iou[i] > NMS_T) & (idxs > i)
        keep = keep & ~sup

    # --- compact kept detections to the front, zero-pad rest
    order = np.argsort(np.where(keep, 0, 1), kind="stable")
    kmask = keep[order]
    out_boxes = np.where(kmask[:, None], top_boxes[order], np.float32(0.0))
    out_scores = np.where(kmask, top_scores[order], np.float32(0.0))
    out_classes = np.where(kmask, top_classes[order], -1).astype(np.int32)
    count = np.int32(keep.sum())
    return out_boxes, out_scores, out_classes, count


def kernel(box_pred, box_conf, box_prob, priors, img_shape):
    box_pred = np.asarray(box_pred, dtype=np.float32)
    box_conf = np.asarray(box_conf, dtype=np.float32)
    box_prob = np.asarray(box_prob, dtype=np.float32)
    priors = np.asarray(priors, dtype=np.float32)
    img_shape = np.asarray(img_shape, dtype=np.float32)

    scores, _ = _device_scores(box_conf, box_prob)
    return _postprocess(scores, box_pred, box_conf, box_prob, priors, img_shape)
